# revision 11
# baseline (speedup 1.0000x reference)
"""TRN2 Bass kernel for nn_ClassicalSelfAttention (N=8192, D=1024) on 8 NeuronCores.

Math: out = softmax((X R)(X E)^T / sqrt(D)) X
    = softmax(X W X^T / sqrt(D)) X with W = R E^T folded on the host.

Row-sharded over 8 cores (m = rows of the query/output). Per core, logits are
computed in [m, n] layout (m on partitions):
    PT[d, m] = sum_d' W[d', d] Xi^T[d', m]      (3-product fp16 hi/lo, prologue)
    L[m, n]  = sum_d P[m, d] X^T[d, n]
softmax stats run on the free axis (DVE reduce + ACT exp with per-partition
bias and accum_out for the row sum), scores are PE-transposed (fp16) to [n, m]
for the AV matmul: out[m, d] = sum_n escore^T[n, m]^T X[n, d].

Precision (use_fp8=True): the L matmul runs as 1 fp16 head product (xh.ph)
plus two fp8e4 DoubleRow correction products at 2 rows/cycle:
    xl8.p8h  ~= xl.P   (xl = X^T - fp16(X^T), scaled 2^12 / 2^-12)
    x8.p8l   ~= X.pl   (pl = P - fp16(P), scaled 2^-2 / 2^2)
giving ~2e-5-relative logits at ~2/3 the PE cost of the fp16 3-product
scheme. The two fp8 operand streams are packed into one [D, 2, N] dram
tensor so each DMA line is 1 KiB. AV uses single fp16.
"""

import numpy as np
import ml_dtypes

import concourse.bass as bass
import concourse.mybir as mybir
import concourse.tile as tile
from concourse.masks import make_identity

N = 8192
D = 1024
NCORES = 8
M = N // NCORES  # 1024 rows per core
P = 128
KO = D // P  # 8 contraction chunks of 128
G = 256  # m-group rows (2 psum row-tiles)
NG = M // G  # 4 groups
NT = 16  # n-tiles of 512 per sweep
NCH = 64  # n-chunks of 128
SCALE = 1.0 / 32.0  # 1/sqrt(D)

# fp8 split scales; each pair multiplies to 1.
S_XL = 4096.0  # xl8 = fp8(xl * 4096)
S_PH = 1.0 / 4096.0  # p8h = fp8(P / 4096)
S_X8 = 0.25  # x8 = fp8(x / 4)
S_PL = 4.0  # p8l = fp8(pl * 4)

F32 = mybir.dt.float32
F16 = mybir.dt.float16
F8 = mybir.dt.float8e4
NPF8 = ml_dtypes.float8_e4m3
DR = mybir.MatmulPerfMode.DoubleRow


def _split_f16(x: np.ndarray):
    hi = x.astype(np.float16)
    lo = (x - hi.astype(np.float32)).astype(np.float16)
    return hi, lo


def _split_waits(nc, max_waits: int = 1):
    """walrus in this toolchain fits only ~1 embedded sync-wait per
    instruction; hoist extras onto standalone NoOps on the same engine."""
    ctr = 0
    for fn in nc.m.functions:
        for bb in fn.blocks:
            insts = list(bb.instructions)
            out = []
            changed = False
            for inst in insts:
                si = getattr(inst, "sync_info", None)
                waits = list(si.on_wait) if si is not None and si.on_wait else []
                if len(waits) > max_waits:
                    changed = True
                    hoist, keep = waits[:-max_waits], waits[-max_waits:]
                    for i in range(0, len(hoist), max_waits):
                        nop = mybir.InstNoOp(name=f"I-waitsplit-{ctr}")
                        ctr += 1
                        nop.engine = inst.engine
                        nop.sync_info = mybir.SyncInfo(
                            on_wait=hoist[i : i + max_waits], on_update=[]
                        )
                        out.append(nop)
                    inst.sync_info = mybir.SyncInfo(
                        on_wait=keep, on_update=list(si.on_update)
                    )
                out.append(inst)
            if changed:
                bb.instructions = out
    return nc


def _mm3(nc, ps, lhs_hi, lhs_lo, rhs_hi, rhs_lo, k_range, first, last):
    """Accumulate the 3-product hi/lo split into psum `ps`."""
    n = len(k_range)
    for i, k in enumerate(k_range):
        nc.tensor.matmul(
            ps, lhs_hi(k), rhs_hi(k), start=(first and i == 0), stop=False
        )
        nc.tensor.matmul(ps, lhs_hi(k), rhs_lo(k), start=False, stop=False)
        nc.tensor.matmul(
            ps, lhs_lo(k), rhs_hi(k), start=False, stop=(last and i == n - 1)
        )


def build_nc(
    split_waits: bool = True,
    reps: int = 1,
    mode: str = "3p",  # "3p" | "x8" | "dr"
    lt_bufs: int = 2,
    xt_bufs: int = 3,
    x_bufs: int = 6,
    e_bufs: int = 6,
    st_bufs: int = 2,
    sT_bufs: int = 8,
    out_bufs: int = 2,
    tr_bufs: int = 2,
):
    nc = bass.Bass("TRN2", target_bir_lowering=False)
    x_d = nc.dram_tensor("x", [N, D], F16, kind="ExternalInput").ap()
    xth_d = nc.dram_tensor("xth", [D, N], F16, kind="ExternalInput").ap()
    if mode == "dr":
        # per 512-wide n-block: [xl8 block | x8 block] so DMA lines are 1 KiB
        x8p_d = nc.dram_tensor("x8p", [D, 2 * N], F8, kind="ExternalInput").ap()
    elif mode == "x8":
        xl8_d = nc.dram_tensor("xl8", [D, N], F8, kind="ExternalInput").ap()
    else:
        xtl_d = nc.dram_tensor("xtl", [D, N], F16, kind="ExternalInput").ap()
    wh_d = nc.dram_tensor("wh", [D, D], F16, kind="ExternalInput").ap()
    wl_d = nc.dram_tensor("wl", [D, D], F16, kind="ExternalInput").ap()
    xith_d = nc.dram_tensor("xith", [D, M], F16, kind="ExternalInput").ap()
    xitl_d = nc.dram_tensor("xitl", [D, M], F16, kind="ExternalInput").ap()
    out_d = nc.dram_tensor("out", [M, D], F32, kind="ExternalOutput").ap()

    def r3(ap):  # [D, W] dram -> [128, KO, W]
        return ap.rearrange("(ko p) w -> p ko w", p=P)

    def r4(ap):  # [D, 2N] dram -> [128, KO, NT, 1024]
        return ap.rearrange("(ko p) (nt x) -> p ko nt x", p=P, x=1024)

    with tile.TileContext(nc) as tc:
        with tc.tile_pool(name="pers", bufs=1) as pers:
            pth = pers.tile([P, KO, M], F16, name="pth")
            ptl = pers.tile([P, KO, M], F16, name="ptl")
            if mode in ("dr", "x8"):
                p8h = pers.tile([P, KO, M], F8, name="p8h")
            if mode == "dr":
                p8l = pers.tile([P, KO, M], F8, name="p8l")
            ident = pers.tile([P, P], F16, name="ident")
            make_identity(nc, ident)

            # ---------------- prologue: PT = W^T Xi^T ----------------
            with (
                tc.tile_pool(name="pro", bufs=1) as pro,
                tc.tile_pool(name="pro_ps", bufs=4, space="PSUM") as pro_ps,
            ):
                wh = pro.tile([P, KO, D], F16, name="wh")
                wl = pro.tile([P, KO, D], F16, name="wl")
                xith = pro.tile([P, KO, M], F16, name="xith")
                xitl = pro.tile([P, KO, M], F16, name="xitl")
                for t, d in (
                    (wh, wh_d), (wl, wl_d), (xith, xith_d), (xitl, xitl_d),
                ):
                    nc.sync.dma_start(t, r3(d))

                for do in range(KO):
                    ds = slice(do * P, (do + 1) * P)
                    for mh2 in range(2):
                        ms = slice(mh2 * 512, (mh2 + 1) * 512)
                        ps = pro_ps.tile([P, 512], F32, name="pro_psum")
                        _mm3(
                            nc, ps,
                            lambda k, ds=ds: wh[:, k, ds],
                            lambda k, ds=ds: wl[:, k, ds],
                            lambda k, ms=ms: xith[:, k, ms],
                            lambda k, ms=ms: xitl[:, k, ms],
                            range(KO), True, True,
                        )
                        nc.scalar.copy(pth[:, do, ms], ps)
                        nc.vector.tensor_tensor(
                            ptl[:, do, ms], ps, pth[:, do, ms],
                            mybir.AluOpType.subtract,
                        )
                        if mode in ("dr", "x8"):
                            nc.scalar.mul(p8h[:, do, ms], ps, S_PH)
                        if mode == "dr":
                            nc.scalar.mul(p8l[:, do, ms], ptl[:, do, ms], S_PL)

            # ---------------- main loop over m-groups ----------------
            with (
                tc.tile_pool(name="lbuf", bufs=1) as lpool,
                tc.tile_pool(name="xts", bufs=xt_bufs) as xt_pool,
                tc.tile_pool(name="x8s", bufs=xt_bufs) as x8_pool,
                tc.tile_pool(name="xs", bufs=x_bufs) as x_pool,
                tc.tile_pool(name="es", bufs=e_bufs) as e_pool,
                tc.tile_pool(name="sts", bufs=st_bufs) as st_pool,
                tc.tile_pool(name="sTs", bufs=sT_bufs) as sT_pool,
                tc.tile_pool(name="outs", bufs=out_bufs) as out_pool,
                tc.tile_pool(name="lt_ps", bufs=lt_bufs, space="PSUM") as lt_ps,
                tc.tile_pool(name="av_ps", bufs=1, space="PSUM") as av_ps,
                tc.tile_pool(name="tr_ps", bufs=tr_bufs, space="PSUM") as tr_ps,
            ):
                l_sb = lpool.tile([P, 2, NT, 512], F32, name="l_sb")

                def emit_group(g):
                    rmp = st_pool.tile([P, 2, NT], F32, name="rmp")
                    sap = st_pool.tile([P, 2, NT], F32, name="sap")
                    bias_t = st_pool.tile([P, 2], F32, name="bias_t")
                    av = [
                        av_ps.tile([P, D], F32, name=f"av{mh}") for mh in range(2)
                    ]
                    # --- LT sweep: logits [m, n] ---
                    for jt in range(NT):
                        ns = slice(jt * 512, (jt + 1) * 512)
                        xth_t = xt_pool.tile([P, KO, 512], F16, name="xth_t")
                        nc.sync.dma_start(xth_t, r3(xth_d)[:, :, ns])
                        if mode == "dr":
                            x8_t = x8_pool.tile([P, KO, 2, 512], F8, name="x8_t")
                            nc.sync.dma_start(x8_t, r4(x8p_d)[:, :, jt])
                        elif mode == "x8":
                            if jt % 2 == 0:
                                ns2 = slice(jt * 512, (jt + 2) * 512)
                                xl8_t = x8_pool.tile(
                                    [P, KO, 2, 512], F8, name="xl8_t"
                                )
                                nc.sync.dma_start(
                                    xl8_t,
                                    r3(xl8_d)[:, :, ns2].rearrange(
                                        "p ko (two w) -> p ko two w", two=2
                                    ),
                                )
                        else:
                            xtl_t = x8_pool.tile([P, KO, 512], F16, name="xtl_t")
                            nc.sync.dma_start(xtl_t, r3(xtl_d)[:, :, ns])
                        for mh in range(2):
                            gs = slice(g * G + mh * P, g * G + (mh + 1) * P)
                            ps = lt_ps.tile([P, 512], F32, name="lt_psum")
                            if mode == "dr":
                                for k in range(KO):
                                    nc.tensor.matmul(
                                        ps, pth[:, k, gs], xth_t[:, k],
                                        start=(k == 0), stop=False,
                                    )
                                for kp in range(KO // 2):
                                    nc.tensor.matmul(
                                        ps,
                                        p8h[:, 2 * kp : 2 * kp + 2, gs],
                                        x8_t[:, 2 * kp : 2 * kp + 2, 0],
                                        start=False, stop=False, perf_mode=DR,
                                    )
                                for kp in range(KO // 2):
                                    nc.tensor.matmul(
                                        ps,
                                        p8l[:, 2 * kp : 2 * kp + 2, gs],
                                        x8_t[:, 2 * kp : 2 * kp + 2, 1],
                                        start=False, stop=(kp == KO // 2 - 1),
                                        perf_mode=DR,
                                    )
                            elif mode == "x8":
                                for k in range(KO):
                                    nc.tensor.matmul(
                                        ps, pth[:, k, gs], xth_t[:, k],
                                        start=(k == 0), stop=False,
                                    )
                                    nc.tensor.matmul(
                                        ps, ptl[:, k, gs], xth_t[:, k],
                                        start=False, stop=False,
                                    )
                                for k in range(KO):
                                    nc.tensor.matmul(
                                        ps,
                                        p8h[:, k, gs],
                                        xl8_t[:, k, jt % 2],
                                        start=False, stop=(k == KO - 1),
                                    )
                            else:
                                _mm3(
                                    nc, ps,
                                    lambda k, gs=gs: pth[:, k, gs],
                                    lambda k, gs=gs: ptl[:, k, gs],
                                    lambda k, xth_t=xth_t: xth_t[:, k],
                                    lambda k, xtl_t=xtl_t: xtl_t[:, k],
                                    range(KO), True, True,
                                )
                            nc.vector.tensor_copy(l_sb[:, mh, jt], ps)
                            nc.vector.tensor_reduce(
                                rmp[:, mh, jt : jt + 1], ps,
                                axis=mybir.AxisListType.X, op=mybir.AluOpType.max,
                            )
                    # --- stats: -max * SCALE as exp bias (free-axis only) ---
                    for mh in range(2):
                        nc.vector.tensor_reduce(
                            bias_t[:, mh : mh + 1], rmp[:, mh],
                            axis=mybir.AxisListType.X, op=mybir.AluOpType.max,
                            negate=True,
                        )
                    nc.scalar.mul(bias_t, bias_t, SCALE)
                    # --- exp + transpose + AV (AV pipelined one step behind
                    # the transposes so PE never waits on the sT copies) ---
                    pend = None  # (sTs, x_t, j)

                    def emit_av(pend):
                        sTs, x_t, j = pend
                        for mh in range(2):
                            for dh in range(2):
                                nc.tensor.matmul(
                                    av[mh][:, dh * 512 : (dh + 1) * 512],
                                    sTs[mh],
                                    x_t[:, dh * 512 : (dh + 1) * 512],
                                    start=(j == 0), stop=(j == NCH - 1),
                                )

                    for nt in range(NT):
                        e_ts = []
                        for mh in range(2):
                            e_t = e_pool.tile([P, 512], F16, name=f"e_t{mh}")
                            nc.scalar.activation(
                                e_t, l_sb[:, mh, nt],
                                mybir.ActivationFunctionType.Exp,
                                bias=bias_t[:, mh : mh + 1], scale=SCALE,
                                accum_out=sap[:, mh, nt : nt + 1],
                            )
                            e_ts.append(e_t)
                        for js in range(4):
                            j = nt * 4 + js
                            x_t = x_pool.tile([P, D], F16, name="x_t")
                            nc.sync.dma_start(x_t, x_d[j * P : (j + 1) * P, :])
                            sTs = []
                            for mh in range(2):
                                tp = tr_ps.tile([P, P], F16, name="tr_ps")
                                nc.tensor.transpose(
                                    tp, e_ts[mh][:, js * P : (js + 1) * P], ident
                                )
                                sT = sT_pool.tile([P, P], F16, name="sT")
                                nc.scalar.copy(sT, tp)
                                sTs.append(sT)
                            if pend is not None:
                                emit_av(pend)
                            pend = (sTs, x_t, j)
                    emit_av(pend)
                    # --- finalize: divide by row sums, store ---
                    ssum = st_pool.tile([P, 2], F32, name="ssum")
                    rcol = st_pool.tile([P, 2], F32, name="rcol")
                    for mh in range(2):
                        nc.vector.tensor_reduce(
                            ssum[:, mh : mh + 1], sap[:, mh],
                            axis=mybir.AxisListType.X, op=mybir.AluOpType.add,
                        )
                    nc.vector.reciprocal(rcol, ssum)
                    for mh in range(2):
                        o_sb = out_pool.tile([P, D], F32, name="o_sb")
                        nc.vector.tensor_scalar_mul(
                            o_sb, av[mh], rcol[:, mh : mh + 1]
                        )
                        row0 = g * G + mh * P
                        nc.sync.dma_start(out_d[row0 : row0 + P], o_sb)

                if reps == 1:
                    for g in range(NG):
                        emit_group(g)
                else:
                    with tc.For_i(0, reps, 1):
                        for g in range(NG):
                            emit_group(g)

    if split_waits:
        _split_waits(nc)
    return nc


_CACHE = {}


def make_in_maps(x: np.ndarray, rot: np.ndarray, ent: np.ndarray, mode="3p"):
    """Host-side prep + per-core input maps (x, rot, ent are fp32)."""
    w = (rot.astype(np.float64) @ ent.astype(np.float64).T).astype(np.float32)
    wh, wl = _split_f16(w)
    x_r = np.ascontiguousarray(x).astype(np.float16)
    xt = np.ascontiguousarray(x.T)
    xth = xt.astype(np.float16)
    common = {"x": x_r, "xth": xth, "wh": wh, "wl": wl}
    if mode == "dr":
        xl = xt - xth.astype(np.float32)
        x8p = np.empty((D, NT, 2, 512), dtype=NPF8)
        x8p[:, :, 0] = (xl * S_XL).astype(NPF8).reshape(D, NT, 512)
        x8p[:, :, 1] = (xt * S_X8).astype(NPF8).reshape(D, NT, 512)
        common["x8p"] = x8p.reshape(D, 2 * N)
    elif mode == "x8":
        xl = xt - xth.astype(np.float32)
        common["xl8"] = (xl * S_XL).astype(NPF8)
    else:
        common["xtl"] = (xt - xth.astype(np.float32)).astype(np.float16)
    in_maps = []
    for c in range(NCORES):
        cs = slice(c * M, (c + 1) * M)
        xith, xitl = _split_f16(np.ascontiguousarray(xt[:, cs]))
        in_maps.append(dict(common, xith=xith, xitl=xitl))
    return in_maps


def kernel(**inputs) -> np.ndarray:
    from concourse.bass_utils import run_bass_kernel_spmd

    x = np.asarray(inputs["inputs"], dtype=np.float32)
    rot = np.asarray(inputs["rotation"], dtype=np.float32)
    ent = np.asarray(inputs["entangle"], dtype=np.float32)

    in_maps = make_in_maps(x, rot, ent, mode="3p")

    if "nc" not in _CACHE:
        _CACHE["nc"] = build_nc()
    nc = _CACHE["nc"]

    res = run_bass_kernel_spmd(nc, in_maps, core_ids=list(range(NCORES)))
    out = np.concatenate([res.results[c]["out"] for c in range(NCORES)], axis=0)
    return np.ascontiguousarray(out.astype(np.float32))


if __name__ == "__main__":
    rng = np.random.default_rng(0)
    x = rng.standard_normal((N, D)).astype(np.float32)
    r = rng.standard_normal((D, D)).astype(np.float32)
    e = rng.standard_normal((D, D)).astype(np.float32)
    o = kernel(inputs=x, rotation=r, entangle=e)
    print(o.shape, o.dtype, float(np.abs(o).max()))
